# revision 1
# baseline (speedup 1.0000x reference)
"""Trainium2 Bass kernel for nn_AttentionConv (dense_transformer).

Sharding: data-parallel over batch — 8 NeuronCores, one batch image each.

Per-core dataflow (T=3136 tokens = 56x56, C=384, 6 heads x 64):
  - x shipped pre-transposed from host as xT [C, T] bf16.
  - Q path: depthwise 3x3 conv + BN on DVE+GPSIMD via scalar_tensor_tensor
    tap accumulation in [c, h, w] layout (BN affine folded into tap
    weights/bias on host), then Q projection on PE (softmax scale folded
    into wq on host) -> qh^T [o, T].
  - K/V path: stride-2 depthwise conv + BN + projection FUSED into 9
    shifted PE matmuls per output tile: kh^T = sum_tap Wtap^T @ x^T
    (strided access patterns), accumulated in PSUM. BN bias folded into a
    per-o bias applied at PSUM evacuation.
  - Attention per head: scores^T [t, q] = kh^T.T @ qh^T on PE, exp on ACT
    (no max-subtraction: |scores| << 1 by construction), o^T [65, q] =
    [vh | ones]^T @ e^T accumulated over t tiles (ones column yields the
    softmax denominator as psum row 64). Denominator folded to [W/8, 8]
    via a DRAM bounce, reciprocal on DVE at full lane use, broadcast back
    to 64 partitions via a 0-stride DMA, applied during PSUM evacuation.
  - vh^T produced from vh [o, t] via PE transpose (identity shipped).
  - Output projection in [l, o] orientation (o^T slices stationary),
    result DMA'd straight to DRAM rows. b_last added on host.
"""
import sys

sys.path.insert(0, '/opt/trn_rl_repo')

import numpy as np

DIM = 384
HEADS = 6
D = 64
S = 56           # stride-1 spatial side
S2 = 28          # stride-2 spatial side
T = S * S        # 3136
T2 = S2 * S2     # 784
EPS = 1e-5
SCALE = DIM ** -0.5
NCORES = 8
CT = DIM // 128          # 3 channel tiles
NTT = (T2 + 127) // 128  # 7 kv t-tiles (last = 16 rows)
QB = 1024                # attention q band width
# the narrow tail band runs second so its serial denominator chain overlaps
# a dense band instead of dangling at the kernel tail
BANDS = [(0, 1024), (3072, 64), (1024, 1024), (2048, 1024)]

TAPS = [(dy, dx) for dy in (-1, 0, 1) for dx in (-1, 0, 1)]  # k=(dy+1)*3+(dx+1)


def build_program():
    import concourse.mybir as mybir
    from concourse import bacc
    from concourse.tile import TileContext

    dt = mybir.dt
    AF = mybir.ActivationFunctionType
    ALU = mybir.AluOpType

    nc = bacc.Bacc()

    SP = S + 2
    xT = nc.dram_tensor("xT", [DIM, SP * SP], dt.bfloat16,
                        kind="ExternalInput")
    qcp = nc.dram_tensor("qcp", [DIM, 10], dt.float32, kind="ExternalInput")
    wqt = nc.dram_tensor("wqt", [DIM, DIM], dt.bfloat16, kind="ExternalInput")
    # K/V conv runs as diagonal-stationary matmuls: wkvt[c, {k,v}, o] = w^T
    # projection weights; kvs[c, 0:9]/[c, 9:18] = per-channel tap scales;
    # kvb[c, {k,v}] = folded BN bias added to the conv features.
    wkvt = nc.dram_tensor("wkvt", [DIM, 2, DIM], dt.bfloat16,
                          kind="ExternalInput")
    kvs = nc.dram_tensor("kvs", [DIM, 18], dt.float32, kind="ExternalInput")
    kvb = nc.dram_tensor("kvb", [DIM, 2], dt.float32, kind="ExternalInput")
    wlt = nc.dram_tensor("wlt", [DIM, DIM], dt.bfloat16, kind="ExternalInput")
    idin = nc.dram_tensor("idin", [128, 128], dt.bfloat16, kind="ExternalInput")
    out = nc.dram_tensor("out", [T, DIM], dt.float32, kind="ExternalOutput")

    with TileContext(nc) as tc:
        with (
            tc.tile_pool(name="const", bufs=1) as cpool,
            tc.tile_pool(name="work", bufs=1) as wpool,
            tc.tile_pool(name="ework", bufs=3) as epool,
            tc.tile_pool(name="psA", bufs=2, space="PSUM") as psA,
            tc.tile_pool(name="psB", bufs=2, space="PSUM") as psB,
            tc.tile_pool(name="dram", bufs=2, space="DRAM") as dpool,
        ):
            # ---------------- Phase 0: loads ----------------
            # x arrives zero-padded [58, 58] so every conv tap is full-region
            xT_sb = cpool.tile([128, CT, SP, SP], dt.bfloat16)
            qcp_sb = cpool.tile([128, CT, 10], dt.float32)
            wqt_sb = cpool.tile([128, CT, DIM], dt.bfloat16)
            dk_sb = cpool.tile([128, 9 * CT, 128], dt.bfloat16)
            dv_sb = cpool.tile([128, 9 * CT, 128], dt.bfloat16)
            kf_sb = cpool.tile([128, CT, T2], dt.bfloat16)
            vf_sb = cpool.tile([128, CT, T2], dt.bfloat16)
            kvb_sb = cpool.tile([128, CT, 2], dt.float32)
            wlt_sb = cpool.tile([128, CT, DIM], dt.bfloat16)
            ident = cpool.tile([128, 128], dt.bfloat16)

            # load order matches consumption: x + conv params first, then K
            # weights in (ctile, tap) order, then V, then the later-phase
            # weights.
            def csl(c):
                return slice(c * 128, (c + 1) * 128)

            wkvt_sb = cpool.tile([128, CT, 2, DIM], dt.bfloat16)
            kvs_sb = cpool.tile([128, CT, 18], dt.float32)
            nc.sync.dma_start(ident[:], idin[:])
            for c in range(CT):
                nc.sync.dma_start(kvs_sb[:, c, :], kvs[csl(c), :])
                nc.sync.dma_start(qcp_sb[:, c, :], qcp[csl(c), :])
                nc.sync.dma_start(
                    xT_sb[:, c, :, :],
                    xT[csl(c), :].rearrange("p (h w) -> p h w", w=SP))
            for c in range(CT):
                nc.sync.dma_start(wkvt_sb[:, c, :, :], wkvt[csl(c), :, :])
                nc.sync.dma_start(wqt_sb[:, c, :], wqt[csl(c), :])
                nc.sync.dma_start(kvb_sb[:, c, :], kvb[csl(c), :])
                nc.sync.dma_start(wlt_sb[:, c, :], wlt[csl(c), :])

            # diagonal conv stationaries: dk[k,c] = diag(tap scale) built as
            # identity * per-partition scale — dk on DVE (fast 4x path, needed
            # immediately). dv builds are emitted inside phase 1 (ACT, after
            # each ctile's conv temps) so they don't delay the conv merge.
            for c in range(CT):
                for k in range(9):
                    nc.vector.tensor_scalar(
                        out=dk_sb[:, k * CT + c, :], in0=ident[:],
                        scalar1=kvs_sb[:, c, k:k + 1], scalar2=0.0,
                        op0=ALU.mult, op1=ALU.add)

            # persistent activations
            q_feat = cpool.tile([128, CT, T], dt.bfloat16)
            qh_sb = cpool.tile([128, CT, T], dt.bfloat16)
            kh_sb = cpool.tile([128, CT, T2], dt.bfloat16)
            vh_sb = cpool.tile([128, CT, T2], dt.bfloat16)
            vhT_sb = cpool.tile([128, NTT, HEADS * 65], dt.bfloat16)
            o_sb = cpool.tile([128, CT, T], dt.bfloat16)
            den_scr = cpool.tile([128, QB], dt.float32)
            den_fold = cpool.tile([128, QB // 8], dt.float32)
            r_fold = cpool.tile([128, QB // 8], dt.float32)

            # ---- Phase 1: Q depthwise conv + BN on PE as diagonal-stationary
            # matmuls (like K/V): 9 shifted taps accumulate in PSUM, BN bias
            # added at DVE evacuation. dq/dv diag builds are cheap DVE/ACT
            # tensor_scalars over the identity.
            dq_sb = cpool.tile([128, 9 * CT, 128], dt.bfloat16)
            for c in range(CT):
                for k in range(9):
                    nc.vector.tensor_scalar(
                        out=dq_sb[:, k * CT + c, :], in0=ident[:],
                        scalar1=qcp_sb[:, c, k:k + 1], scalar2=0.0,
                        op0=ALU.mult, op1=ALU.add)
                    nc.scalar.activation(
                        dv_sb[:, k * CT + c, :], ident[:],
                        AF.Copy, scale=kvs_sb[:, c, 9 + k:10 + k])

            QROWS = 8  # h-rows per conv chunk: 8*56 = 448 free
            for c in range(CT):
                x3 = xT_sb[:, c, :, :]  # [128, 58, 58] zero-padded
                for r0 in range(0, S, QROWS):
                    ps = psA.tile([128, QB], dt.float32, tag="psA")
                    for k in range(9):
                        dy, dx = TAPS[k]
                        nc.tensor.matmul(
                            ps[:, 0:QROWS * S],
                            dq_sb[:, k * CT + c, :],
                            x3[:, 1 + dy + r0:1 + dy + r0 + QROWS,
                               1 + dx:1 + dx + S],
                            start=(k == 0), stop=(k == 8))
                    nc.vector.tensor_scalar_add(
                        q_feat[:, c, r0 * S:(r0 + QROWS) * S],
                        ps[:, 0:QROWS * S],
                        qcp_sb[:, c, 9:10])

            # ------------- Phase 3: K/V stride-2 conv + projection ----------
            # Conv: 9 accumulating matmuls per (ctile, chunk) with a DIAGONAL
            # stationary (per-channel tap scale) and strided rhs — N cycles
            # per tap instead of 9x'ing the projection FLOPs. BN bias folded
            # in at PSUM evacuation (per-partition = per-channel). Then a
            # plain [C->C] projection.
            def kv_conv(d_sb, f_sb, bias_col):
                for c in range(CT):
                    x5 = xT_sb[:, c, :, :].rearrange(
                        "p (h sy) (w sx) -> p h sy w sx", sy=2, sx=2)
                    for ha, hb in ((0, 14), (14, 28)):
                        ps = psA.tile([128, QB], dt.float32, tag="psA")
                        for k in range(9):
                            dy, dx = TAPS[k]
                            hoff, sy = ((0, 0) if dy == -1 else
                                        (0, 1) if dy == 0 else (1, 0))
                            woff, sx = ((0, 0) if dx == -1 else
                                        (0, 1) if dx == 0 else (1, 0))
                            nc.tensor.matmul(
                                ps[:, 0:(hb - ha) * S2],
                                d_sb[:, k * CT + c, :],
                                x5[:, ha + hoff:hb + hoff, sy,
                                   woff:woff + S2, sx],
                                start=(k == 0), stop=(k == 8))
                        nc.vector.tensor_scalar_add(
                            f_sb[:, c, ha * S2:hb * S2],
                            ps[:, 0:14 * S2],
                            kvb_sb[:, c, bias_col:bias_col + 1])

            def kv_proj(f_sb, dst_sb, wcol):
                for ot in range(CT):
                    osl = slice(ot * 128, (ot + 1) * 128)
                    for ha, hb in ((0, 14), (14, 28)):
                        ps = psA.tile([128, QB], dt.float32, tag="psA")
                        for c in range(CT):
                            nc.tensor.matmul(
                                ps[:, 0:(hb - ha) * S2],
                                wkvt_sb[:, c, wcol, osl],
                                f_sb[:, c, ha * S2:hb * S2],
                                start=(c == 0), stop=(c == CT - 1))
                        nc.vector.tensor_copy(
                            dst_sb[:, ot, ha * S2:hb * S2],
                            ps[:, 0:14 * S2])

            kv_conv(dk_sb, kf_sb, 0)
            kv_proj(kf_sb, kh_sb, 0)
            kv_conv(dv_sb, vf_sb, 1)
            kv_proj(vf_sb, vh_sb, 1)

            # -------- Phase 4: vh^T [t, (head, 65)] with ones column --------
            # contiguous full-tile memset (a strided ones-column memset is
            # priced per-element and would cost ~100us); the data copies
            # below overwrite everything except the ones columns.
            v4 = vhT_sb[:].rearrange("p n (h c) -> p n h c", c=65)
            nc.vector.memset(vhT_sb[:], 1.0)
            for tt in range(NTT):
                tsz = min(128, T2 - tt * 128)
                for ot in range(CT):
                    pst = psB.tile([128, QB], dt.bfloat16, tag="psB")
                    nc.tensor.transpose(
                        pst[0:tsz, 0:128],
                        vh_sb[:, ot, tt * 128:tt * 128 + tsz],
                        ident[:])
                    nc.vector.tensor_copy(
                        v4[0:tsz, tt, 2 * ot:2 * ot + 2, 0:64],
                        pst[0:tsz, 0:128].rearrange("p (h c) -> p h c", c=64))

            # ---------------- Phase 2: Q projection (qh^T [o, T]) -----------
            LCH = 448  # 7 chunks exactly
            for lc in (0, 1, 2, 3, 4, 5, 6):
                lsl = slice(lc * LCH, (lc + 1) * LCH)
                for ot in range(CT):
                    osl = slice(ot * 128, (ot + 1) * 128)
                    ps = psA.tile([128, QB], dt.float32, tag="psA")
                    for c in range(CT):
                        nc.tensor.matmul(
                            ps[:, 0:LCH], wqt_sb[:, c, osl], q_feat[:, c, lsl],
                            start=(c == 0), stop=(c == CT - 1))
                    nc.vector.tensor_copy(qh_sb[:, ot, lsl], ps[:, 0:LCH])

            # ---------------- Phase 5: attention ----------------
            # band-outer / head-inner; the PREVIOUS band's output-projection
            # tiles are spread between this band's heads so PE fills the
            # ACT-bound stretches without starving the scores PSUM slots.
            def oproj_tile(lpos, lsz):
                ps = psB.tile([128, QB], dt.float32, tag="psB")
                for c in range(CT):
                    nc.tensor.matmul(
                        ps[0:lsz, 0:DIM], o_sb[:, c, lpos:lpos + lsz],
                        wlt_sb[:, c, :],
                        start=(c == 0), stop=(c == CT - 1))
                ostage = epool.tile([128, DIM], dt.float32, tag="ostage")
                nc.vector.tensor_copy(ostage[0:lsz, :], ps[0:lsz, 0:DIM])
                nc.sync.dma_start(out[lpos:lpos + lsz, :], ostage[0:lsz, :])

            def band_ltiles(qs, W):
                return [(qs + i, min(128, qs + W - (qs + i)))
                        for i in range(0, W, 128)]

            def head_tloop(h, qs, W, ps_o, obase):
                """scores -> exp -> o accumulation for one head over all
                t-tiles, software-pipelined so PE never stalls on ACT."""
                ot = h // 2
                hsl = slice(64 * (h % 2), 64 * (h % 2) + 64)

                def scores(tt):
                    tsz = min(128, T2 - tt * 128)
                    ps_s = psA.tile([128, QB], dt.float32, tag="psA")
                    for sub in range(0, W, 512):
                        sw = min(512, W - sub)
                        nc.tensor.matmul(
                            ps_s[0:tsz, sub:sub + sw],
                            kh_sb[hsl, ot, tt * 128:tt * 128 + tsz],
                            qh_sb[hsl, ot, qs + sub:qs + sub + sw],
                            start=True, stop=True)
                    return ps_s

                ps_s = scores(0)
                for tt in range(NTT):
                    tsz = min(128, T2 - tt * 128)
                    e = epool.tile([128, QB], dt.bfloat16, tag="e")
                    nc.scalar.activation(e[0:tsz, 0:W], ps_s[0:tsz, 0:W],
                                         AF.Exp)
                    if tt + 1 < NTT:
                        ps_s = scores(tt + 1)
                    for sub in range(0, W, 512):
                        sw = min(512, W - sub)
                        nc.tensor.matmul(
                            ps_o[0:65, obase + sub:obase + sub + sw],
                            vhT_sb[0:tsz, tt, h * 65:h * 65 + 65],
                            e[0:tsz, sub:sub + sw],
                            start=(tt == 0), stop=(tt == NTT - 1))

            def norm_chain(ps_o, WW):
                """den row -> fold via DRAM -> recip -> broadcast r [64, WW]"""
                fw = 8
                fp = WW // fw
                den_dr = dpool.tile([QB], dt.float32, tag="dd")
                r_dr = dpool.tile([QB], dt.float32, tag="rd")
                nc.vector.tensor_copy(den_scr[64:65, 0:WW],
                                      ps_o[64:65, 0:WW])
                nc.sync.dma_start(den_dr[None, 0:WW], den_scr[64:65, 0:WW])
                nc.sync.dma_start(
                    den_fold[0:fp, 0:fw],
                    den_dr[0:WW].rearrange("(p f) -> p f", f=fw))
                nc.vector.reciprocal(r_fold[0:fp, 0:fw],
                                     den_fold[0:fp, 0:fw])
                nc.sync.dma_start(
                    r_dr[0:WW].rearrange("(p f) -> p f", f=fw),
                    r_fold[0:fp, 0:fw])
                r_rep = epool.tile([64, QB], dt.float32, tag="r_rep")
                nc.sync.dma_start(r_rep[0:64, 0:WW],
                                  r_dr[None, 0:WW].to_broadcast([64, WW]))
                return r_rep

            def evac_head(h, qs, W, ps_o, obase, r_rep, rbase):
                ot = h // 2
                hsl = slice(64 * (h % 2), 64 * (h % 2) + 64)
                nc.vector.tensor_tensor(
                    out=o_sb[hsl, ot, qs:qs + W],
                    in0=ps_o[0:64, obase:obase + W],
                    in1=r_rep[0:64, rbase:rbase + W],
                    op=ALU.mult)

            prev_band = None
            for qs, W in BANDS:
                if W * HEADS <= 512:
                    # narrow tail band: all heads share one PSUM tile and a
                    # single denominator chain.
                    ps_o = psB.tile([128, QB], dt.float32, tag="psB")
                    for h in range(HEADS):
                        head_tloop(h, qs, W, ps_o, h * W)
                        if prev_band is not None:
                            tiles = band_ltiles(*prev_band)
                            if h < len(tiles):
                                oproj_tile(*tiles[h])
                    r_rep = norm_chain(ps_o, W * HEADS)
                    for h in range(HEADS):
                        evac_head(h, qs, W, ps_o, h * W, r_rep, h * W)
                else:
                    for h in range(HEADS):
                        ps_o = psB.tile([128, QB], dt.float32, tag="psB")
                        head_tloop(h, qs, W, ps_o, 0)
                        r_rep = norm_chain(ps_o, W)
                        evac_head(h, qs, W, ps_o, 0, r_rep, 0)
                        if prev_band is not None:
                            tiles = band_ltiles(*prev_band)
                            if h < len(tiles):
                                oproj_tile(*tiles[h])

                if prev_band is not None:
                    for lt in band_ltiles(*prev_band)[HEADS:]:
                        oproj_tile(*lt)
                prev_band = (qs, W)

            for lt in band_ltiles(*prev_band):
                oproj_tile(*lt)

    nc.compile()
    return nc


_CACHE = {}


def _prep_weights(inputs):
    import ml_dtypes
    bf16 = ml_dtypes.bfloat16
    f32 = np.float32

    def bn_fold(prefix):
        a = (np.asarray(inputs[f'bn{prefix}_s'], f32)
             / np.sqrt(np.asarray(inputs[f'bn{prefix}_v'], f32) + EPS))
        b = (np.asarray(inputs[f'bn{prefix}_b'], f32)
             - np.asarray(inputs[f'bn{prefix}_m'], f32) * a)
        return a.astype(f32), b.astype(f32)

    aq, bq = bn_fold('q')
    ak, bk = bn_fold('k')
    av, bv = bn_fold('v')

    conv_q = np.asarray(inputs['conv_q'], f32)[:, 0].reshape(DIM, 9)
    conv_k = np.asarray(inputs['conv_k'], f32)[:, 0].reshape(DIM, 9)
    conv_v = np.asarray(inputs['conv_v'], f32)[:, 0].reshape(DIM, 9)
    wq = np.asarray(inputs['wq'], f32)
    wk = np.asarray(inputs['wk'], f32)
    wv = np.asarray(inputs['wv'], f32)
    wl = np.asarray(inputs['w_last'], f32)

    qcp = np.zeros((DIM, 10), f32)
    qcp[:, :9] = conv_q * aq[:, None]
    qcp[:, 9] = bq

    wqt = np.ascontiguousarray((wq * SCALE).T).astype(bf16)  # [c, o]

    wkvt = np.stack([wk.T, wv.T], axis=1).astype(bf16)  # [c, {k,v}, o]
    kvs = np.concatenate([conv_k * ak[:, None], conv_v * av[:, None]],
                         axis=1).astype(f32)            # [c, 18]
    kvb = np.stack([bk, bv], axis=1).astype(f32)        # [c, 2]
    wlt = np.ascontiguousarray(wl.T).astype(bf16)
    idin = np.eye(128, dtype=bf16)
    return qcp, wqt, wkvt, kvs, kvb, wlt, idin


def _prep_x(xb):
    """[T, C] f32 -> zero-padded transposed [C, 58*58] bf16."""
    import ml_dtypes
    pad = np.zeros((DIM, S + 2, S + 2), np.float32)
    pad[:, 1:1 + S, 1:1 + S] = xb.T.reshape(DIM, S, S)
    return pad.reshape(DIM, (S + 2) * (S + 2)).astype(ml_dtypes.bfloat16)


def kernel(**inputs):
    from concourse.bass_utils import run_bass_kernel_spmd
    import ml_dtypes
    bf16 = ml_dtypes.bfloat16

    if 'nc' not in _CACHE:
        _CACHE['nc'] = build_program()
    nc = _CACHE['nc']

    qcp, wqt, wkvt, kvs, kvb, wlt, idin = _prep_weights(inputs)
    x = np.asarray(inputs['x'], np.float32)  # [8, T, C]
    B = x.shape[0]

    in_maps = []
    for b in range(B):
        in_maps.append({
            'xT': _prep_x(x[b]), 'qcp': qcp, 'wqt': wqt, 'wkvt': wkvt,
            'kvs': kvs, 'kvb': kvb, 'wlt': wlt, 'idin': idin,
        })

    res = run_bass_kernel_spmd(nc, in_maps, list(range(NCORES)))
    outs = np.stack([np.asarray(res.results[b]['out']) for b in range(B)],
                    axis=0)
    outs = outs + np.asarray(inputs['b_last'], np.float32)[None, None, :]
    return outs.astype(np.float32)

